# revision 1
# baseline (speedup 1.0000x reference)
"""HGT (heterogeneous graph transformer) 2-layer forward on 8 Trainium2 cores.

Strategy (edge/dst sharding):
 - Nodes are permuted and dealt to 8 cores (1250 drug + 1250 disease + 3750
   protein each, padded to 1280/1280/3840 = 6400 slots so every 128-node tile
   is type-pure). Each core owns the edges whose dst lands in its slice.
 - Node front (per core): k/v projections for all 51200 node slots into a
   DRAM [k|v] table (bf16 512B rows, gathered by src); qa = x @ (Wq .
   blockdiag(rel_att^T) . pri/sqrt(D)) per etype for local nodes (gathered
   by dst).
 - Edge phase: edges grouped by (src-half, etype, dst-degree-bucket) into
   128-edge tiles where a dst node's edges occupy d consecutive partitions.
   Per tile: gather kv+qa rows (int16 dma_gather on 4 SWDGE queues),
   a = sum16(ke*qa), ea = exp(a)*mask, W = [v*ea | ea]; a constant block-sum
   matmul S_d^T @ W reduces each node's edges; partials dma_scatter_add into
   an hv table at row node*4+etype (unique rows per half; lo/hi halves
   serialized by a fence so CCE adds never race).
 - Node epilogue: stream hv rows (no gather), esum over etypes, divide,
   transpose, o^T = sum_g M2[g,t]^T @ hvn_g^T with M2 = blockdiag(rel_msg)@Wa,
   sigmoid-skip blend, write x_new^T.
 - One bf16 AllGather of x_new between the layers; final slices assembled on
   the host.
"""
import numpy as np
import ml_dtypes

H, D, IN = 8, 16, 128
NT, ET = 3, 4
N_DRUG, N_DIS, N_PROT = 10000, 10000, 30000
N = N_DRUG + N_DIS + N_PROT
E = 400000
NCORES = 8
SQRT_D = 4.0

TCNT = (N_DRUG // NCORES, N_DIS // NCORES, N_PROT // NCORES)
TPAD = tuple(-(-c // 128) * 128 for c in TCNT)          # 1280,1280,3840
NLOC = sum(TPAD)                                        # 6400
NSLOT = NCORES * NLOC                                   # 51200
LO_LIMIT = 32768
HVROW = 192                                             # f32 elems per hv row
HVE = 136                                               # used elems
HV_ROWS = 4 * NLOC + 128                                # + junk strip
JUNK = 4 * NLOC
CN = 8                                                  # tiles per gather call
SC_B = 8                                                # batches per scatter
EPS_ESUM = 1e-30
DCLS = (1, 2, 4, 8, 16, 32, 64, 128)

BF16 = ml_dtypes.bfloat16


def _bf(x):
    return np.ascontiguousarray(np.asarray(x).astype(BF16))


# ---------------------------------------------------------------- host prep
def _host_prep(inp):
    src = np.asarray(inp["src"]).astype(np.int64)
    dst = np.asarray(inp["dst"]).astype(np.int64)
    etype = np.asarray(inp["etype"]).astype(np.int64)
    x0 = np.concatenate([np.asarray(inp["drug_feature"]),
                         np.asarray(inp["disease_feature"]),
                         np.asarray(inp["protein_feature"])], 0).astype(np.float32)

    Wk = np.asarray(inp["Wk"], np.float32)
    Wq = np.asarray(inp["Wq"], np.float32)
    Wv = np.asarray(inp["Wv"], np.float32)
    Wa = np.asarray(inp["Wa"], np.float32)
    rel_att = np.asarray(inp["rel_att"], np.float32)
    rel_msg = np.asarray(inp["rel_msg"], np.float32)
    pri = np.asarray(inp["pri"], np.float32)
    skip = np.asarray(inp["skip"], np.float32)

    BDQ = np.zeros((ET, IN, IN), np.float32)
    BDM = np.zeros((ET, IN, IN), np.float32)
    for g in range(ET):
        for h in range(H):
            sl = slice(h * D, (h + 1) * D)
            BDQ[g][sl, sl] = rel_att[h, g].T * (pri[h, g] / SQRT_D)
            BDM[g][sl, sl] = rel_msg[h, g]
    QAW = np.einsum("tio,goj->tgij", Wq, BDQ)
    M2 = np.einsum("gij,tjo->gtio", BDM, Wa)
    alpha = 1.0 / (1.0 + np.exp(-skip))

    # node -> slot assignment
    deg_tot = np.bincount(dst, minlength=N)
    old_of_slot = np.full(NSLOT, -1, np.int64)
    tbase = (0, TPAD[0], TPAD[0] + TPAD[1])
    for t, (lo, cnt) in enumerate(
            zip((0, N_DRUG, N_DRUG + N_DIS), (N_DRUG, N_DIS, N_PROT))):
        ids = np.arange(lo, lo + cnt)
        ids = ids[np.argsort(-deg_tot[ids], kind="stable")]
        percore = [[] for _ in range(NCORES)]
        for i, nid in enumerate(ids):
            r, k = divmod(i, NCORES)
            c = k if r % 2 == 0 else NCORES - 1 - k
            percore[c].append(nid)
        for c in range(NCORES):
            arr = np.sort(np.array(percore[c], np.int64))
            o = c * NLOC + tbase[t]
            old_of_slot[o:o + len(arr)] = arr
    slot_of_old = np.full(N, -1, np.int64)
    real = old_of_slot >= 0
    slot_of_old[old_of_slot[real]] = np.nonzero(real)[0]

    xs = np.zeros((NSLOT, IN), np.float32)
    xs[real] = x0[old_of_slot[real]]

    slot_t = np.zeros(NLOC, np.int64)
    slot_t[TPAD[0]:TPAD[0] + TPAD[1]] = 1
    slot_t[TPAD[0] + TPAD[1]:] = 2

    # edges in slot space
    e_src = slot_of_old[src]
    e_loc_all = slot_of_old[dst]
    e_core = e_loc_all // NLOC
    e_loc = e_loc_all % NLOC
    e_half = (e_src >= LO_LIMIT).astype(np.int64)

    # group edges by (core, half, g, local dst) -> degree buckets
    node_entries = [[[{d: [] for d in DCLS} for _ in range(ET)] for _ in range(2)]
                    for _ in range(NCORES)]
    order = np.lexsort((e_loc, etype, e_half, e_core))
    oc, oh, og, ol = (e_core[order], e_half[order], etype[order], e_loc[order])
    i, M = 0, len(order)
    while i < M:
        j = i
        c, hf, g, n = oc[i], oh[i], og[i], ol[i]
        while j < M and oc[j] == c and oh[j] == hf and og[j] == g and ol[j] == n:
            j += 1
        k = j - i
        assert k <= 128, f"degree {k} > 128 unsupported"
        d = next(dd for dd in DCLS if dd >= k)
        node_entries[c][hf][g][d].append((n, order[i:j]))
        i = j

    ntile = {}
    for hf in range(2):
        for g in range(ET):
            for d in DCLS:
                cap = 128 // d
                mx = max(len(node_entries[c][hf][g][d]) for c in range(NCORES))
                ntile[(hf, g, d)] = -(-mx // cap) if mx else 0

    tiles = []
    for hf in range(2):
        for g in range(ET):
            for d in DCLS:
                tiles += [(hf, g, d)] * ntile[(hf, g, d)]
    T = len(tiles)
    n_lo_tiles = sum(1 for t in tiles if t[0] == 0)

    kv_idx = np.zeros((NCORES, T, 128), np.int32)
    qa_idx = np.zeros((NCORES, T, 128), np.int32)
    mask = np.zeros((NCORES, T, 128), np.float32)
    sc_row = np.full((NCORES, T, 128), JUNK, np.int32)

    for c in range(NCORES):
        ti = 0
        for hf in range(2):
            for g in range(ET):
                for d in DCLS:
                    nt = ntile[(hf, g, d)]
                    if nt == 0:
                        continue
                    cap = 128 // d
                    ents = node_entries[c][hf][g][d]
                    for tt in range(nt):
                        for s, (n, eids) in enumerate(
                                ents[tt * cap:(tt + 1) * cap]):
                            sc_row[c, ti + tt, s] = n * 4 + g
                            p0 = s * d
                            for k2, eid in enumerate(eids):
                                kv_idx[c, ti + tt, p0 + k2] = (
                                    e_src[eid] - (LO_LIMIT if hf else 0))
                                qa_idx[c, ti + tt, p0 + k2] = (
                                    g * NLOC + e_loc[eid])
                                mask[c, ti + tt, p0 + k2] = 1.0
                    ti += nt
        assert ti == T

    # gather calls (never cross the lo/hi boundary)
    def _chunks(lo, hi):
        return [(t0, min(t0 + CN, hi)) for t0 in range(lo, hi, CN)]
    g_calls = _chunks(0, n_lo_tiles) + _chunks(n_lo_tiles, T)

    # PSUM batches
    batches = []
    i = 0
    while i < T:
        hf, g, d = tiles[i]
        if d == 1:
            batches.append({"d1": True, "tiles": [i]})
            i += 1
            continue
        per = 2 if d == 2 else 3
        grp = []
        while len(grp) < per and i < T and tiles[i] == (hf, g, d):
            grp.append(i)
            i += 1
        step = 64 if d == 2 else 32
        batches.append({"d1": False, "d": d, "tiles": grp, "step": step,
                        "qoffs": [k * step for k in range(len(grp))]})
    NB = len(batches)

    def bat_half(bi):
        return tiles[batches[bi]["tiles"][0]][0]

    sc_calls = []
    i = 0
    while i < NB:
        j = i
        while j < NB and j - i < SC_B and bat_half(j) == bat_half(i):
            j += 1
        sc_calls.append((i, j))
        i = j

    sc_idx = np.full((NCORES, NB, 128), JUNK, np.int32)
    for c in range(NCORES):
        for bi, b in enumerate(batches):
            if b["d1"]:
                sc_idx[c, bi, :] = sc_row[c, b["tiles"][0], :]
            else:
                d, step = b["d"], b["step"]
                m = 128 // d
                for (tidx, qoff) in zip(b["tiles"], b["qoffs"]):
                    sc_idx[c, bi, qoff:qoff + m] = sc_row[c, tidx, :m]

    def wrap16(flat):
        a = flat.astype(np.int16).reshape(-1, 16).T
        return np.tile(a, (8, 1))

    def build_gather_idx(arr):
        return np.concatenate(
            [wrap16(arr[t0:t1].reshape(-1)) for (t0, t1) in g_calls], 1)

    kv_idx_w = np.stack([build_gather_idx(kv_idx[c]) for c in range(NCORES)])
    qa_idx_w = np.stack([build_gather_idx(qa_idx[c]) for c in range(NCORES)])
    sc_idx_w = np.stack([
        np.concatenate([wrap16(sc_idx[c, b0:b1].reshape(-1))
                        for (b0, b1) in sc_calls], 1)
        for c in range(NCORES)])
    mask_bf = np.stack([_bf(mask[c].T) for c in range(NCORES)])

    # const block
    blocks, cmap = [], {}

    def add_c(name, mat):
        cmap[name] = (sum(b.shape[1] for b in blocks), mat.shape[1])
        blocks.append(_bf(mat))
    for t in range(NT):
        add_c(f"Wk{t}", Wk[t])
        add_c(f"Wv{t}", Wv[t])
    for t in range(NT):
        for g in range(ET):
            add_c(f"QAW{t}{g}", QAW[t, g])
    for g in range(ET):
        for t in range(NT):
            add_c(f"M2{g}{t}", M2[g, t])
    for d in DCLS[1:]:
        m = 128 // d
        w = 64 if d == 2 else 32
        S = np.zeros((128, w), np.float32)
        for s in range(m):
            S[s * d:(s + 1) * d, s] = 1.0
        add_c(f"S{d}", S)
    add_c("Szero", np.zeros((128, 64), np.float32))
    add_c("ident", np.eye(128, dtype=np.float32))
    wconst = np.concatenate(blocks, 1)

    meta = {"tiles": tiles, "g_calls": g_calls, "batches": batches,
            "sc_calls": sc_calls, "n_lo_tiles": n_lo_tiles, "cmap": cmap,
            "wcols": wconst.shape[1], "slot_t": slot_t, "alpha": alpha,
            "T": T, "NB": NB}
    percore = {"kv_idx": kv_idx_w, "qa_idx": qa_idx_w, "sc_idx": sc_idx_w,
               "mask": mask_bf,
               "xTloc": np.stack([_bf(xs[c * NLOC:(c + 1) * NLOC].T)
                                  for c in range(NCORES)])}
    shared = {"xT": np.concatenate(
                  [_bf(xs[c * NLOC:(c + 1) * NLOC].T) for c in range(NCORES)], 0),
              "wconst": wconst}
    asm = {"old_of_slot": old_of_slot, "real": real}
    return meta, percore, shared, asm


# ---------------------------------------------------------------- bass build
def _build(meta, last_layer, repeats=1):
    import contextlib
    import concourse.bacc as bacc
    import concourse.mybir as mybir
    import concourse.tile as tile
    from concourse import library_config
    from concourse.tile_rust import add_dep_helper

    f32 = mybir.dt.float32
    bf16 = mybir.dt.bfloat16
    i16 = mybir.dt.int16
    AX = mybir.AxisListType.X
    AF = mybir.ActivationFunctionType

    tiles = meta["tiles"]
    g_calls = meta["g_calls"]
    batches = meta["batches"]
    sc_calls = meta["sc_calls"]
    cmap = meta["cmap"]
    WCOLS = meta["wcols"]
    slot_t = meta["slot_t"]
    alpha = meta["alpha"]
    T = meta["T"]
    NB = meta["NB"]

    IDXW = sum(8 * (t1 - t0) for (t0, t1) in g_calls)
    SCW = NB * 8
    NST = NLOC // 512 + (1 if NLOC % 512 else 0)

    nc = bacc.Bacc("TRN2", target_bir_lowering=False, debug=False,
                   num_swdge_queues=1)

    xT_in = nc.dram_tensor("xT", [NCORES * IN, NLOC], bf16, kind="ExternalInput")
    xTloc_in = nc.dram_tensor("xTloc", [IN, NLOC], bf16, kind="ExternalInput")
    wconst_in = nc.dram_tensor("wconst", [128, WCOLS], bf16, kind="ExternalInput")
    kvidx_in = nc.dram_tensor("kvidx", [128, IDXW], i16, kind="ExternalInput")
    qaidx_in = nc.dram_tensor("qaidx", [128, IDXW], i16, kind="ExternalInput")
    scidx_in = nc.dram_tensor("scidx", [128, SCW], i16, kind="ExternalInput")
    mask_in = nc.dram_tensor("mask", [128, T], bf16, kind="ExternalInput")
    out_dt = f32 if last_layer else bf16
    out_t = nc.dram_tensor("out", [IN, NLOC], out_dt, kind="ExternalOutput")

    kv_tbl = nc.dram_tensor("kv_tbl", [NSLOT, 2 * IN], bf16)
    qa_tbl = nc.dram_tensor("qa_tbl", [ET * NLOC, IN], bf16)
    hv1 = nc.dram_tensor("hv1", [HV_ROWS, HVROW], f32)

    with tile.TileContext(nc) as tc, contextlib.ExitStack() as ctx:
        lib_inst = nc.gpsimd.load_library(library_config.mlp)

        consts = ctx.enter_context(tc.tile_pool(name="consts", bufs=1))
        wsb = consts.tile([128, WCOLS], bf16)
        nc.sync.dma_start(out=wsb[:, :], in_=wconst_in[:, :])

        def cst(name):
            off, w = cmap[name]
            return wsb[:, off:off + w]

        idx_kv = consts.tile([128, IDXW], i16)
        idx_qa = consts.tile([128, IDXW], i16)
        idx_sc = consts.tile([128, SCW], i16)
        msk = consts.tile([128, T], bf16)
        nc.sync.dma_start(out=idx_kv[:, :], in_=kvidx_in[:, :])
        nc.sync.dma_start(out=idx_qa[:, :], in_=qaidx_in[:, :])
        nc.sync.dma_start(out=idx_sc[:, :], in_=scidx_in[:, :])
        nc.sync.dma_start(out=msk[:, :], in_=mask_in[:, :])

        zt = consts.tile([128, HVROW], f32)
        nc.vector.memset(zt[:, :], 0.0)

        def zero_hv(hv):
            import concourse.bass as bass
            insts = []
            step = 4096
            for r in range(0, HV_ROWS, step):
                n = min(step, HV_ROWS - r)
                z2 = zt[:, :]
                src = bass.AP(tensor=z2.tensor, offset=z2.offset,
                              ap=[list(z2.ap[0]), [0, n // 128],
                                  list(z2.ap[1])])
                insts.append(nc.gpsimd.dma_start(
                    out=hv[r:r + n, :].rearrange("(p a) f -> p a f", a=n // 128),
                    in_=src))
            return insts

        def fence(producers):
            nop = nc.sync.nop()
            for p in producers:
                add_dep_helper(nop.ins, p.ins, reason="fb")
            return nop

        def gate(consumer, nop):
            if nop is not None:
                add_dep_helper(consumer.ins, nop.ins, reason="ff")

        import concourse.bass as bass

        def expand_inner(a, count):
            return bass.AP(tensor=a.tensor, offset=a.offset,
                           ap=[list(x) for x in a.ap] + [[0, count]])

        def cpy(alt, out, in_):
            if alt % 2:
                return nc.scalar.copy(out=out, in_=in_)
            return nc.vector.tensor_copy(out=out, in_=in_)

        zh1 = zero_hv(hv1)

        # ---------------- node front
        def node_front(xsrc_ap, xloc_tensor, gnop, writes):
            with tc.tile_pool(name="nf", bufs=4) as nf, \
                 tc.tile_pool(name="nfp", bufs=2, space="PSUM") as nfp:
                for c2 in range(NCORES):
                    for st in range(NST):
                        w = min(512, NLOC - st * 512)
                        ntl = w // 128
                        xin = nf.tile([128, 512], bf16, tag="xin")
                        ld = nc.sync.dma_start(
                            out=xin[:, :w],
                            in_=xsrc_ap[c2 * IN:(c2 + 1) * IN,
                                        st * 512:st * 512 + w])
                        gate(ld, gnop)
                        kps = nfp.tile([128, 4, 128], f32, tag="kps")
                        vps = nfp.tile([128, 4, 128], f32, tag="vps")
                        for jt in range(ntl):
                            t = int(slot_t[(st * 4 + jt) * 128])
                            nc.tensor.matmul(
                                out=kps[:, jt, :],
                                lhsT=xin[:, jt * 128:(jt + 1) * 128],
                                rhs=cst(f"Wk{t}"), start=True, stop=True)
                            nc.tensor.matmul(
                                out=vps[:, jt, :],
                                lhsT=xin[:, jt * 128:(jt + 1) * 128],
                                rhs=cst(f"Wv{t}"), start=True, stop=True)
                        ksb = nf.tile([128, 4, 128], bf16, tag="ksb")
                        vsb = nf.tile([128, 4, 128], bf16, tag="vsb")
                        nc.scalar.copy(out=ksb[:, :ntl, :], in_=kps[:, :ntl, :])
                        nc.vector.tensor_copy(out=vsb[:, :ntl, :],
                                              in_=vps[:, :ntl, :])
                        base = c2 * NLOC + st * 512
                        writes.append(nc.sync.dma_start(
                            out=kv_tbl[base:base + w, 0:IN].rearrange(
                                "(j p) f -> p j f", p=128),
                            in_=ksb[:, :ntl, :]))
                        writes.append(nc.scalar.dma_start(
                            out=kv_tbl[base:base + w, IN:2 * IN].rearrange(
                                "(j p) f -> p j f", p=128),
                            in_=vsb[:, :ntl, :]))
                # qa (local)
                for st in range(NST):
                    w = min(512, NLOC - st * 512)
                    ntl = w // 128
                    xin = nf.tile([128, 512], bf16, tag="xin")
                    ld = nc.sync.dma_start(
                        out=xin[:, :w],
                        in_=xloc_tensor[:, st * 512:st * 512 + w])
                    gate(ld, gnop)
                    for g in range(ET):
                        qps = nfp.tile([128, 4, 128], f32, tag="kps")
                        for jt in range(ntl):
                            t = int(slot_t[(st * 4 + jt) * 128])
                            nc.tensor.matmul(
                                out=qps[:, jt, :],
                                lhsT=xin[:, jt * 128:(jt + 1) * 128],
                                rhs=cst(f"QAW{t}{g}"), start=True, stop=True)
                        qsb = nf.tile([128, 4, 128], bf16, tag="ksb")
                        cpy(g, qsb[:, :ntl, :], qps[:, :ntl, :])
                        base = g * NLOC + st * 512
                        writes.append(nc.sync.dma_start(
                            out=qa_tbl[base:base + w, :].rearrange(
                                "(j p) f -> p j f", p=128),
                            in_=qsb[:, :ntl, :]))

        # ---------------- edge phase
        def edge_phase(hv, nf_nop, zh_insts):
            sc_lo, sc_hi = [], []
            with tc.tile_pool(name="eg", bufs=4) as eg, \
                 tc.tile_pool(name="ew", bufs=40) as ew, \
                 tc.tile_pool(name="est", bufs=5) as est, \
                 tc.tile_pool(name="stgp", bufs=3) as stgp, \
                 tc.tile_pool(name="epsum", bufs=2, space="PSUM") as epsum:

                W_tiles = {}
                bat_cursor = [0]
                stg_state = {"tile": None, "k": 0, "b0": 0, "insts": []}
                sc_ci = [0]

                def flush_scatter():
                    st = stg_state
                    if st["tile"] is None or st["k"] == 0:
                        return
                    b0, b1 = sc_calls[sc_ci[0]]
                    assert b1 - b0 == st["k"], (b0, b1, st["k"])
                    nb = st["k"]
                    si = nc.gpsimd.dma_scatter_add(
                        hv[:, :HVE], st["tile"][:, :nb, :],
                        idx_sc[:, b0 * 8:b0 * 8 + nb * 8],
                        128 * nb, 128 * nb, HVE, elem_step=HVROW,
                        queue_num=0)
                    add_dep_helper(si.ins, lib_inst.ins, reason="lib")
                    for z in zh_insts:
                        add_dep_helper(si.ins, z.ins, reason="zh")
                    (sc_lo if tiles[batches[b0]["tiles"][0]][0] == 0
                     else sc_hi).append(si)
                    sc_ci[0] += 1
                    st["tile"] = None
                    st["k"] = 0

                def process_ready_batches(tiles_done):
                    while bat_cursor[0] < NB:
                        b = batches[bat_cursor[0]]
                        if b["tiles"][-1] >= tiles_done:
                            return
                        st = stg_state
                        if st["tile"] is None:
                            st["tile"] = stgp.tile([128, SC_B, HVE], f32,
                                                   name="stg", tag="stg")
                        k = st["k"]
                        if b["d1"]:
                            ti = b["tiles"][0]
                            cpy(k, st["tile"][:, k, :], W_tiles[ti][:, :])
                            del W_tiles[ti]
                        else:
                            d, step = b["d"], b["step"]
                            ps = epsum.tile([128, HVE], f32, tag="ps")
                            for (tidx, qoff) in zip(b["tiles"], b["qoffs"]):
                                nc.tensor.matmul(
                                    out=ps[qoff:qoff + step, :],
                                    lhsT=cst(f"S{d}")[:, :step],
                                    rhs=W_tiles[tidx][:, :],
                                    start=True, stop=True)
                            nstrip = 2 if b["d"] == 2 else 3
                            for k2 in range(len(b["tiles"]), nstrip):
                                nc.tensor.matmul(
                                    out=ps[k2 * step:(k2 + 1) * step, :],
                                    lhsT=cst("Szero")[:, :step],
                                    rhs=W_tiles[b["tiles"][-1]][:, :],
                                    start=True, stop=True)
                            if b["d"] >= 4:
                                nc.vector.memset(ps[96:128, :], 0.0)
                            for tidx in b["tiles"]:
                                del W_tiles[tidx]
                            cpy(k, st["tile"][:, k, :], ps[:, :])
                        st["k"] += 1
                        bat_cursor[0] += 1
                        b0, b1 = sc_calls[sc_ci[0]]
                        if st["k"] == b1 - b0:
                            flush_scatter()

                idx_off = 0
                for ci, (t0, t1) in enumerate(g_calls):
                    ntl = t1 - t0
                    lo = tiles[t0][0] == 0
                    kvb = eg.tile([128, CN, 2 * IN], bf16, tag="kvb")
                    qab = eg.tile([128, CN, IN], bf16, tag="qab")
                    src_ap = (kv_tbl[0:LO_LIMIT, :] if lo
                              else kv_tbl[LO_LIMIT:NSLOT, :])
                    gi = nc.gpsimd.dma_gather(
                        kvb[:, :ntl, :], src_ap,
                        idx_kv[:, idx_off:idx_off + ntl * 8],
                        128 * ntl, 128 * ntl, 2 * IN, queue_num=0)
                    gate(gi, nf_nop)
                    add_dep_helper(gi.ins, lib_inst.ins, reason="lib")
                    gq = nc.gpsimd.dma_gather(
                        qab[:, :ntl, :], qa_tbl[:, :],
                        idx_qa[:, idx_off:idx_off + ntl * 8],
                        128 * ntl, 128 * ntl, IN, queue_num=0)
                    gate(gq, nf_nop)
                    add_dep_helper(gq.ins, lib_inst.ins, reason="lib")
                    idx_off += ntl * 8

                    ast = est.tile([128, CN, H], f32, tag="ast")
                    for j in range(ntl):
                        tmp = est.tile([128, 128], bf16, tag="tmp")
                        nc.vector.tensor_mul(out=tmp[:, :],
                                             in0=kvb[:, j, 0:IN],
                                             in1=qab[:, j, :])
                        nc.vector.reduce_sum(
                            out=ast[:, j, :],
                            in_=tmp[:, :].rearrange("p (h d) -> p h d", h=H),
                            axis=AX)
                    eat = est.tile([128, CN, H], bf16, tag="eat")
                    nc.scalar.activation(out=eat[:, :ntl, :],
                                         in_=ast[:, :ntl, :], func=AF.Exp,
                                         scale=1.0)
                    for j in range(ntl):
                        ti = t0 + j
                        Wt = ew.tile([128, HVE], bf16, tag="wt")
                        nc.vector.tensor_mul(
                            out=Wt[:, IN:HVE], in0=eat[:, j, :],
                            in1=msk[:, ti:ti + 1].broadcast_to([128, H]))
                        nc.vector.tensor_mul(
                            out=Wt[:, 0:IN].rearrange("p (h d) -> p h d", h=H),
                            in0=kvb[:, j, IN:2 * IN].rearrange(
                                "p (h d) -> p h d", h=H),
                            in1=expand_inner(Wt[:, IN:HVE], D))
                        W_tiles[ti] = Wt
                    process_ready_batches(t1)
                process_ready_batches(T + 1)
                flush_scatter()
                assert bat_cursor[0] == NB and not W_tiles
            # lo/hi serialization for hv scatter-adds
            if sc_lo and sc_hi:
                fn = fence(sc_lo)
                for si in sc_hi:
                    gate(si, fn)
            return sc_lo + sc_hi

        # ---------------- epilogue
        def epilogue(hv, xloc_tensor, out_tensor, out_dtype, sc_nop):
            outs = []
            with tc.tile_pool(name="epi", bufs=4) as epi, \
                 tc.tile_pool(name="epp", bufs=2, space="PSUM") as epp:
                for ch in range(NLOC // 128):
                    hvt = epi.tile([128, 4, HVROW], f32, tag="hvt")
                    ld = nc.sync.dma_start(
                        out=hvt[:, :, :],
                        in_=hv[ch * 512:(ch + 1) * 512, :].rearrange(
                            "(p g) f -> p g f", g=4))
                    gate(ld, sc_nop)
                    xl = epi.tile([128, 128], bf16, tag="xl")
                    ldx = nc.sync.dma_start(
                        out=xl[:, :],
                        in_=xloc_tensor[:, ch * 128:(ch + 1) * 128])
                    gate(ldx, sc_nop)
                    es = epi.tile([128, H], f32, tag="es")
                    e2 = epi.tile([128, H], f32, tag="e2")
                    nc.vector.tensor_add(out=es[:, :], in0=hvt[:, 0, IN:HVE],
                                         in1=hvt[:, 1, IN:HVE])
                    nc.vector.tensor_add(out=e2[:, :], in0=hvt[:, 2, IN:HVE],
                                         in1=hvt[:, 3, IN:HVE])
                    nc.vector.tensor_add(out=es[:, :], in0=es[:, :],
                                         in1=e2[:, :])
                    nc.vector.tensor_scalar_add(out=es[:, :], in0=es[:, :],
                                                scalar1=EPS_ESUM)
                    nc.vector.reciprocal(out=es[:, :], in_=es[:, :])
                    t = int(slot_t[ch * 128])
                    ops = epp.tile([128, 128], f32, tag="ops")
                    for g in range(ET):
                        hvn = epi.tile([128, 128], bf16, tag="hvn")
                        nc.vector.tensor_mul(
                            out=hvn[:, :].rearrange("p (h d) -> p h d", h=H),
                            in0=hvt[:, g, 0:IN].rearrange(
                                "p (h d) -> p h d", h=H),
                            in1=expand_inner(es[:, :], D))
                        tp = epp.tile([128, 128], bf16, tag="tp")
                        nc.tensor.transpose(out=tp[:, :], in_=hvn[:, :],
                                            identity=cst("ident"))
                        hvnT = epi.tile([128, 128], bf16, tag="hvnT")
                        cpy(g, hvnT[:, :], tp[:, :])
                        nc.tensor.matmul(out=ops[:, :], lhsT=cst(f"M2{g}{t}"),
                                         rhs=hvnT[:, :], start=(g == 0),
                                         stop=(g == 3))
                    dd = epi.tile([128, 128], f32, tag="dd")
                    nc.vector.tensor_sub(out=dd[:, :], in0=ops[:, :],
                                         in1=xl[:, :])
                    nc.vector.tensor_scalar_mul(out=dd[:, :], in0=dd[:, :],
                                                scalar1=float(alpha[t]))
                    ot = epi.tile([128, 128], out_dtype, tag="ot")
                    nc.vector.tensor_add(out=ot[:, :], in0=dd[:, :],
                                         in1=xl[:, :])
                    outs.append(nc.sync.dma_start(
                        out=out_tensor[:, ch * 128:(ch + 1) * 128],
                        in_=ot[:, :]))
            return outs

        # ================= single layer (optionally repeated for timing)
        prev = None
        for rep in range(repeats):
            if rep > 0:
                zh1 = zero_hv(hv1)
                for z in zh1:
                    gate(z, prev)
            writes1 = []
            node_front(xT_in[:, :], xTloc_in, prev, writes1)
            f1 = fence(writes1)
            sc1 = edge_phase(hv1, f1, zh1)
            f3 = fence(sc1)
            eps = epilogue(hv1, xTloc_in, out_t, out_dt, f3)
            prev = fence(eps)

    nc.compile()
    return nc


# ---------------------------------------------------------------- runner
def _in_maps(meta, percore, shared, xT, xTloc_percore):
    maps = []
    for c in range(NCORES):
        maps.append({
            "xT": xT, "xTloc": xTloc_percore[c],
            "wconst": shared["wconst"], "kvidx": percore["kv_idx"][c],
            "qaidx": percore["qa_idx"][c], "scidx": percore["sc_idx"][c],
            "mask": percore["mask"][c]})
    return maps


def kernel(**inputs) -> np.ndarray:
    from concourse import bass2jax

    meta, percore, shared, asm = _host_prep(inputs)
    nc1 = _build(meta, last_layer=False)
    nc2 = _build(meta, last_layer=True)

    maps1 = _in_maps(meta, percore, shared, shared["xT"], percore["xTloc"])
    res1 = bass2jax.run_bass_via_pjrt(nc1, maps1, n_cores=NCORES)
    xnew = [np.asarray(res1[c]["out"]) for c in range(NCORES)]   # (128,6400) bf16
    xag = np.concatenate(xnew, 0)                                # (1024,6400)

    maps2 = _in_maps(meta, percore, shared, xag, xnew)
    res2 = bass2jax.run_bass_via_pjrt(nc2, maps2, n_cores=NCORES)

    out = np.zeros((N, IN), np.float32)
    for c in range(NCORES):
        oc = np.asarray(res2[c]["out"]).T
        sl = slice(c * NLOC, (c + 1) * NLOC)
        rl = asm["real"][sl]
        out[asm["old_of_slot"][sl][rl]] = oc[rl]
    return out



# revision 2
# speedup vs baseline: 1.5103x; 1.5103x over previous
"""HGT (heterogeneous graph transformer) 2-layer forward on 8 Trainium2 cores.

Strategy (edge/dst sharding):
 - Nodes are permuted and dealt to 8 cores (1250 drug + 1250 disease + 3750
   protein each, padded to 1280/1280/3840 = 6400 slots so every 128-node tile
   is type-pure). Each core owns the edges whose dst lands in its slice.
 - Node front (per core): k/v projections for all 51200 node slots into a
   DRAM [k|v] table (bf16 512B rows, gathered by src); qa = x @ (Wq .
   blockdiag(rel_att^T) . pri/sqrt(D)) per etype for local nodes (gathered
   by dst).
 - Edge phase: edges grouped by (src-half, etype, dst-degree-bucket) into
   128-edge tiles where a dst node's edges occupy d consecutive partitions.
   Per tile: gather kv+qa rows (int16 dma_gather on 4 SWDGE queues),
   a = sum16(ke*qa), ea = exp(a)*mask, W = [v*ea | ea]; a constant block-sum
   matmul S_d^T @ W reduces each node's edges; partials dma_scatter_add into
   an hv table at row node*4+etype (unique rows per half; lo/hi halves
   serialized by a fence so CCE adds never race).
 - Node epilogue: stream hv rows (no gather), esum over etypes, divide,
   transpose, o^T = sum_g M2[g,t]^T @ hvn_g^T with M2 = blockdiag(rel_msg)@Wa,
   sigmoid-skip blend, write x_new^T.
 - One bf16 AllGather of x_new between the layers; final slices assembled on
   the host.
"""
import numpy as np
import ml_dtypes

H, D, IN = 8, 16, 128
NT, ET = 3, 4
N_DRUG, N_DIS, N_PROT = 10000, 10000, 30000
N = N_DRUG + N_DIS + N_PROT
E = 400000
NCORES = 8
SQRT_D = 4.0

TCNT = (N_DRUG // NCORES, N_DIS // NCORES, N_PROT // NCORES)
TPAD = tuple(-(-c // 128) * 128 for c in TCNT)          # 1280,1280,3840
NLOC = sum(TPAD)                                        # 6400
NSLOT = NCORES * NLOC                                   # 51200
LO_LIMIT = 32768
HVROW = 192                                             # f32 elems per hv row
HVE = 136                                               # used elems
HV_ROWS = 4 * NLOC + 128                                # + junk strip
JUNK = 4 * NLOC
CN = 8                                                  # tiles per gather call
SC_B = 8                                                # batches per scatter
EPS_ESUM = 1e-30
DCLS = (1, 2, 4, 8, 16, 32, 64, 128)

BF16 = ml_dtypes.bfloat16


def _bf(x):
    return np.ascontiguousarray(np.asarray(x).astype(BF16))


# ---------------------------------------------------------------- host prep
def _host_prep(inp):
    src = np.asarray(inp["src"]).astype(np.int64)
    dst = np.asarray(inp["dst"]).astype(np.int64)
    etype = np.asarray(inp["etype"]).astype(np.int64)
    x0 = np.concatenate([np.asarray(inp["drug_feature"]),
                         np.asarray(inp["disease_feature"]),
                         np.asarray(inp["protein_feature"])], 0).astype(np.float32)

    Wk = np.asarray(inp["Wk"], np.float32)
    Wq = np.asarray(inp["Wq"], np.float32)
    Wv = np.asarray(inp["Wv"], np.float32)
    Wa = np.asarray(inp["Wa"], np.float32)
    rel_att = np.asarray(inp["rel_att"], np.float32)
    rel_msg = np.asarray(inp["rel_msg"], np.float32)
    pri = np.asarray(inp["pri"], np.float32)
    skip = np.asarray(inp["skip"], np.float32)

    BDQ = np.zeros((ET, IN, IN), np.float32)
    BDM = np.zeros((ET, IN, IN), np.float32)
    for g in range(ET):
        for h in range(H):
            sl = slice(h * D, (h + 1) * D)
            BDQ[g][sl, sl] = rel_att[h, g].T * (pri[h, g] / SQRT_D)
            BDM[g][sl, sl] = rel_msg[h, g]
    QAW = np.einsum("tio,goj->tgij", Wq, BDQ)
    M2 = np.einsum("gij,tjo->gtio", BDM, Wa)
    alpha = 1.0 / (1.0 + np.exp(-skip))

    # node -> slot assignment
    deg_tot = np.bincount(dst, minlength=N)
    old_of_slot = np.full(NSLOT, -1, np.int64)
    tbase = (0, TPAD[0], TPAD[0] + TPAD[1])
    for t, (lo, cnt) in enumerate(
            zip((0, N_DRUG, N_DRUG + N_DIS), (N_DRUG, N_DIS, N_PROT))):
        ids = np.arange(lo, lo + cnt)
        ids = ids[np.argsort(-deg_tot[ids], kind="stable")]
        percore = [[] for _ in range(NCORES)]
        for i, nid in enumerate(ids):
            r, k = divmod(i, NCORES)
            c = k if r % 2 == 0 else NCORES - 1 - k
            percore[c].append(nid)
        for c in range(NCORES):
            arr = np.sort(np.array(percore[c], np.int64))
            o = c * NLOC + tbase[t]
            old_of_slot[o:o + len(arr)] = arr
    slot_of_old = np.full(N, -1, np.int64)
    real = old_of_slot >= 0
    slot_of_old[old_of_slot[real]] = np.nonzero(real)[0]

    xs = np.zeros((NSLOT, IN), np.float32)
    xs[real] = x0[old_of_slot[real]]

    slot_t = np.zeros(NLOC, np.int64)
    slot_t[TPAD[0]:TPAD[0] + TPAD[1]] = 1
    slot_t[TPAD[0] + TPAD[1]:] = 2

    # edges in slot space
    e_src = slot_of_old[src]
    e_loc_all = slot_of_old[dst]
    e_core = e_loc_all // NLOC
    e_loc = e_loc_all % NLOC
    e_half = (e_src >= LO_LIMIT).astype(np.int64)

    # group edges by (core, half, g, local dst) -> degree buckets
    node_entries = [[[{d: [] for d in DCLS} for _ in range(ET)] for _ in range(2)]
                    for _ in range(NCORES)]
    order = np.lexsort((e_loc, etype, e_half, e_core))
    oc, oh, og, ol = (e_core[order], e_half[order], etype[order], e_loc[order])
    i, M = 0, len(order)
    while i < M:
        j = i
        c, hf, g, n = oc[i], oh[i], og[i], ol[i]
        while j < M and oc[j] == c and oh[j] == hf and og[j] == g and ol[j] == n:
            j += 1
        k = j - i
        assert k <= 128, f"degree {k} > 128 unsupported"
        d = next(dd for dd in DCLS if dd >= k)
        node_entries[c][hf][g][d].append((n, order[i:j]))
        i = j

    ntile = {}
    for hf in range(2):
        for g in range(ET):
            for d in DCLS:
                cap = 128 // d
                mx = max(len(node_entries[c][hf][g][d]) for c in range(NCORES))
                ntile[(hf, g, d)] = -(-mx // cap) if mx else 0

    tiles = []
    for hf in range(2):
        for g in range(ET):
            for d in DCLS:
                tiles += [(hf, g, d)] * ntile[(hf, g, d)]
    T = len(tiles)
    n_lo_tiles = sum(1 for t in tiles if t[0] == 0)

    kv_idx = np.zeros((NCORES, T, 128), np.int32)
    qa_idx = np.zeros((NCORES, T, 128), np.int32)
    mask = np.zeros((NCORES, T, 128), np.float32)
    sc_row = np.full((NCORES, T, 128), JUNK, np.int32)

    for c in range(NCORES):
        ti = 0
        for hf in range(2):
            for g in range(ET):
                for d in DCLS:
                    nt = ntile[(hf, g, d)]
                    if nt == 0:
                        continue
                    cap = 128 // d
                    ents = node_entries[c][hf][g][d]
                    for tt in range(nt):
                        for s, (n, eids) in enumerate(
                                ents[tt * cap:(tt + 1) * cap]):
                            sc_row[c, ti + tt, s] = n * 4 + g
                            p0 = s * d
                            for k2, eid in enumerate(eids):
                                kv_idx[c, ti + tt, p0 + k2] = (
                                    e_src[eid] - (LO_LIMIT if hf else 0))
                                qa_idx[c, ti + tt, p0 + k2] = (
                                    g * NLOC + e_loc[eid])
                                mask[c, ti + tt, p0 + k2] = 1.0
                    ti += nt
        assert ti == T

    # gather calls (never cross the lo/hi boundary)
    def _chunks(lo, hi):
        return [(t0, min(t0 + CN, hi)) for t0 in range(lo, hi, CN)]
    g_calls = _chunks(0, n_lo_tiles) + _chunks(n_lo_tiles, T)

    # PSUM batches
    batches = []
    i = 0
    while i < T:
        hf, g, d = tiles[i]
        if d == 1:
            batches.append({"d1": True, "tiles": [i]})
            i += 1
            continue
        per = 2 if d == 2 else 3
        grp = []
        while len(grp) < per and i < T and tiles[i] == (hf, g, d):
            grp.append(i)
            i += 1
        step = 64 if d == 2 else 32
        batches.append({"d1": False, "d": d, "tiles": grp, "step": step,
                        "qoffs": [k * step for k in range(len(grp))]})
    NB = len(batches)

    def bat_half(bi):
        return tiles[batches[bi]["tiles"][0]][0]

    sc_calls = []
    i = 0
    while i < NB:
        j = i
        while j < NB and j - i < SC_B and bat_half(j) == bat_half(i):
            j += 1
        sc_calls.append((i, j))
        i = j

    sc_idx = np.full((NCORES, NB, 128), JUNK, np.int32)
    for c in range(NCORES):
        for bi, b in enumerate(batches):
            if b["d1"]:
                sc_idx[c, bi, :] = sc_row[c, b["tiles"][0], :]
            else:
                d, step = b["d"], b["step"]
                m = 128 // d
                for (tidx, qoff) in zip(b["tiles"], b["qoffs"]):
                    sc_idx[c, bi, qoff:qoff + m] = sc_row[c, tidx, :m]

    def wrap16(flat):
        a = flat.astype(np.int16).reshape(-1, 16).T
        return np.tile(a, (8, 1))

    def build_gather_idx(arr):
        return np.concatenate(
            [wrap16(arr[t0:t1].reshape(-1)) for (t0, t1) in g_calls], 1)

    kv_idx_w = np.stack([build_gather_idx(kv_idx[c]) for c in range(NCORES)])
    qa_idx_w = np.stack([build_gather_idx(qa_idx[c]) for c in range(NCORES)])
    sc_idx_w = np.stack([
        np.concatenate([wrap16(sc_idx[c, b0:b1].reshape(-1))
                        for (b0, b1) in sc_calls], 1)
        for c in range(NCORES)])
    mask_bf = np.stack([_bf(mask[c].T) for c in range(NCORES)])

    # const block
    blocks, cmap = [], {}

    def add_c(name, mat):
        cmap[name] = (sum(b.shape[1] for b in blocks), mat.shape[1])
        blocks.append(_bf(mat))
    for t in range(NT):
        add_c(f"Wk{t}", Wk[t])
        add_c(f"Wv{t}", Wv[t])
    for t in range(NT):
        for g in range(ET):
            add_c(f"QAW{t}{g}", QAW[t, g])
    for g in range(ET):
        for t in range(NT):
            add_c(f"M2{g}{t}", M2[g, t])
    for d in DCLS[1:]:
        m = 128 // d
        w = 64 if d == 2 else 32
        S = np.zeros((128, w), np.float32)
        for s in range(m):
            S[s * d:(s + 1) * d, s] = 1.0
        add_c(f"S{d}", S)
    add_c("Szero", np.zeros((128, 64), np.float32))
    add_c("ident", np.eye(128, dtype=np.float32))
    wconst = np.concatenate(blocks, 1)

    meta = {"tiles": tiles, "g_calls": g_calls, "batches": batches,
            "sc_calls": sc_calls, "n_lo_tiles": n_lo_tiles, "cmap": cmap,
            "wcols": wconst.shape[1], "slot_t": slot_t, "alpha": alpha,
            "T": T, "NB": NB}
    percore = {"kv_idx": kv_idx_w, "qa_idx": qa_idx_w, "sc_idx": sc_idx_w,
               "mask": mask_bf,
               "xTloc": np.stack([_bf(xs[c * NLOC:(c + 1) * NLOC].T)
                                  for c in range(NCORES)])}
    shared = {"xT": np.concatenate(
                  [_bf(xs[c * NLOC:(c + 1) * NLOC].T) for c in range(NCORES)], 0),
              "wconst": wconst}
    asm = {"old_of_slot": old_of_slot, "real": real}
    return meta, percore, shared, asm


# ---------------------------------------------------------------- bass build
def _build(meta, last_layer, repeats=1):
    import contextlib
    import concourse.bacc as bacc
    import concourse.mybir as mybir
    import concourse.tile as tile
    from concourse import library_config
    from concourse.tile_rust import add_dep_helper

    f32 = mybir.dt.float32
    bf16 = mybir.dt.bfloat16
    i16 = mybir.dt.int16
    AX = mybir.AxisListType.X
    AF = mybir.ActivationFunctionType

    tiles = meta["tiles"]
    g_calls = meta["g_calls"]
    batches = meta["batches"]
    sc_calls = meta["sc_calls"]
    cmap = meta["cmap"]
    WCOLS = meta["wcols"]
    slot_t = meta["slot_t"]
    alpha = meta["alpha"]
    T = meta["T"]
    NB = meta["NB"]

    IDXW = sum(8 * (t1 - t0) for (t0, t1) in g_calls)
    SCW = NB * 8
    NST = NLOC // 512 + (1 if NLOC % 512 else 0)

    nc = bacc.Bacc("TRN2", target_bir_lowering=False, debug=False,
                   num_swdge_queues=1, dynamic_dma_scratch_size=65536)

    xT_in = nc.dram_tensor("xT", [NCORES * IN, NLOC], bf16, kind="ExternalInput")
    xTloc_in = nc.dram_tensor("xTloc", [IN, NLOC], bf16, kind="ExternalInput")
    wconst_in = nc.dram_tensor("wconst", [128, WCOLS], bf16, kind="ExternalInput")
    kvidx_in = nc.dram_tensor("kvidx", [128, IDXW], i16, kind="ExternalInput")
    qaidx_in = nc.dram_tensor("qaidx", [128, IDXW], i16, kind="ExternalInput")
    scidx_in = nc.dram_tensor("scidx", [128, SCW], i16, kind="ExternalInput")
    mask_in = nc.dram_tensor("mask", [128, T], bf16, kind="ExternalInput")
    out_dt = f32 if last_layer else bf16
    out_t = nc.dram_tensor("out", [IN, NLOC], out_dt, kind="ExternalOutput")

    kv_tbl = nc.dram_tensor("kv_tbl", [NSLOT, 2 * IN], bf16)
    qa_tbl = nc.dram_tensor("qa_tbl", [ET * NLOC, IN], bf16)
    hv1 = nc.dram_tensor("hv1", [HV_ROWS, HVROW], f32)

    with tile.TileContext(nc) as tc, contextlib.ExitStack() as ctx:
        lib_inst = nc.gpsimd.load_library(library_config.mlp)

        consts = ctx.enter_context(tc.tile_pool(name="consts", bufs=1))
        wsb = consts.tile([128, WCOLS], bf16)
        nc.sync.dma_start(out=wsb[:, :], in_=wconst_in[:, :])

        def cst(name):
            off, w = cmap[name]
            return wsb[:, off:off + w]

        idx_kv = consts.tile([128, IDXW], i16)
        idx_qa = consts.tile([128, IDXW], i16)
        idx_sc = consts.tile([128, SCW], i16)
        msk = consts.tile([128, T], bf16)
        nc.sync.dma_start(out=idx_kv[:, :], in_=kvidx_in[:, :])
        nc.sync.dma_start(out=idx_qa[:, :], in_=qaidx_in[:, :])
        nc.sync.dma_start(out=idx_sc[:, :], in_=scidx_in[:, :])
        nc.sync.dma_start(out=msk[:, :], in_=mask_in[:, :])

        zt = consts.tile([128, HVROW], f32)
        nc.vector.memset(zt[:, :], 0.0)

        def zero_hv(hv):
            import concourse.bass as bass
            insts = []
            step = 4096
            for r in range(0, HV_ROWS, step):
                n = min(step, HV_ROWS - r)
                z2 = zt[:, :]
                src = bass.AP(tensor=z2.tensor, offset=z2.offset,
                              ap=[list(z2.ap[0]), [0, n // 128],
                                  list(z2.ap[1])])
                insts.append(nc.gpsimd.dma_start(
                    out=hv[r:r + n, :].rearrange("(p a) f -> p a f", a=n // 128),
                    in_=src))
            return insts

        def fence(producers):
            nop = nc.sync.nop()
            for p in producers:
                add_dep_helper(nop.ins, p.ins, reason="fb")
            return nop

        def gate(consumer, nop):
            if nop is not None:
                add_dep_helper(consumer.ins, nop.ins, reason="ff")

        import concourse.bass as bass

        def expand_inner(a, count):
            return bass.AP(tensor=a.tensor, offset=a.offset,
                           ap=[list(x) for x in a.ap] + [[0, count]])

        def cpy(alt, out, in_):
            if alt % 2:
                return nc.scalar.copy(out=out, in_=in_)
            return nc.vector.tensor_copy(out=out, in_=in_)

        zh1 = zero_hv(hv1)

        # ---------------- node front
        def node_front(xsrc_ap, xloc_tensor, gnop, writes):
            with tc.tile_pool(name="nf", bufs=4) as nf, \
                 tc.tile_pool(name="nfp", bufs=2, space="PSUM") as nfp:
                for c2 in range(NCORES):
                    for st in range(NST):
                        w = min(512, NLOC - st * 512)
                        ntl = w // 128
                        xin = nf.tile([128, 512], bf16, tag="xin")
                        ld = nc.sync.dma_start(
                            out=xin[:, :w],
                            in_=xsrc_ap[c2 * IN:(c2 + 1) * IN,
                                        st * 512:st * 512 + w])
                        gate(ld, gnop)
                        kps = nfp.tile([128, 4, 128], f32, tag="kps")
                        vps = nfp.tile([128, 4, 128], f32, tag="vps")
                        for jt in range(ntl):
                            t = int(slot_t[(st * 4 + jt) * 128])
                            nc.tensor.matmul(
                                out=kps[:, jt, :],
                                lhsT=xin[:, jt * 128:(jt + 1) * 128],
                                rhs=cst(f"Wk{t}"), start=True, stop=True)
                            nc.tensor.matmul(
                                out=vps[:, jt, :],
                                lhsT=xin[:, jt * 128:(jt + 1) * 128],
                                rhs=cst(f"Wv{t}"), start=True, stop=True)
                        ksb = nf.tile([128, 4, 128], bf16, tag="ksb")
                        vsb = nf.tile([128, 4, 128], bf16, tag="vsb")
                        nc.scalar.copy(out=ksb[:, :ntl, :], in_=kps[:, :ntl, :])
                        nc.vector.tensor_copy(out=vsb[:, :ntl, :],
                                              in_=vps[:, :ntl, :])
                        base = c2 * NLOC + st * 512
                        writes.append(nc.sync.dma_start(
                            out=kv_tbl[base:base + w, 0:IN].rearrange(
                                "(j p) f -> p j f", p=128),
                            in_=ksb[:, :ntl, :]))
                        writes.append(nc.scalar.dma_start(
                            out=kv_tbl[base:base + w, IN:2 * IN].rearrange(
                                "(j p) f -> p j f", p=128),
                            in_=vsb[:, :ntl, :]))
                # qa (local)
                for st in range(NST):
                    w = min(512, NLOC - st * 512)
                    ntl = w // 128
                    xin = nf.tile([128, 512], bf16, tag="xin")
                    ld = nc.sync.dma_start(
                        out=xin[:, :w],
                        in_=xloc_tensor[:, st * 512:st * 512 + w])
                    gate(ld, gnop)
                    for g in range(ET):
                        qps = nfp.tile([128, 4, 128], f32, tag="kps")
                        for jt in range(ntl):
                            t = int(slot_t[(st * 4 + jt) * 128])
                            nc.tensor.matmul(
                                out=qps[:, jt, :],
                                lhsT=xin[:, jt * 128:(jt + 1) * 128],
                                rhs=cst(f"QAW{t}{g}"), start=True, stop=True)
                        qsb = nf.tile([128, 4, 128], bf16, tag="ksb")
                        cpy(g, qsb[:, :ntl, :], qps[:, :ntl, :])
                        base = g * NLOC + st * 512
                        writes.append(nc.sync.dma_start(
                            out=qa_tbl[base:base + w, :].rearrange(
                                "(j p) f -> p j f", p=128),
                            in_=qsb[:, :ntl, :]))

        # ---------------- edge phase
        def edge_phase(hv, nf_nop, zh_insts):
            sc_lo, sc_hi = [], []
            with tc.tile_pool(name="eg", bufs=4) as eg, \
                 tc.tile_pool(name="ew", bufs=40) as ew, \
                 tc.tile_pool(name="est", bufs=5) as est, \
                 tc.tile_pool(name="stgp", bufs=3) as stgp, \
                 tc.tile_pool(name="epsum", bufs=2, space="PSUM") as epsum:

                W_tiles = {}
                bat_cursor = [0]
                stg_state = {"tile": None, "k": 0, "b0": 0, "insts": []}
                sc_ci = [0]

                def flush_scatter():
                    st = stg_state
                    if st["tile"] is None or st["k"] == 0:
                        return
                    b0, b1 = sc_calls[sc_ci[0]]
                    assert b1 - b0 == st["k"], (b0, b1, st["k"])
                    nb = st["k"]
                    si = nc.gpsimd.dma_scatter_add(
                        hv[:, :HVE], st["tile"][:, :nb, :],
                        idx_sc[:, b0 * 8:b0 * 8 + nb * 8],
                        128 * nb, 128 * nb, HVE, elem_step=HVROW,
                        queue_num=0)
                    add_dep_helper(si.ins, lib_inst.ins, reason="lib")
                    for z in zh_insts:
                        add_dep_helper(si.ins, z.ins, reason="zh")
                    (sc_lo if tiles[batches[b0]["tiles"][0]][0] == 0
                     else sc_hi).append(si)
                    sc_ci[0] += 1
                    st["tile"] = None
                    st["k"] = 0

                def process_ready_batches(tiles_done):
                    while bat_cursor[0] < NB:
                        b = batches[bat_cursor[0]]
                        if b["tiles"][-1] >= tiles_done:
                            return
                        st = stg_state
                        if st["tile"] is None:
                            st["tile"] = stgp.tile([128, SC_B, HVE], f32,
                                                   name="stg", tag="stg")
                        k = st["k"]
                        if b["d1"]:
                            ti = b["tiles"][0]
                            cpy(k, st["tile"][:, k, :], W_tiles[ti][:, :])
                            del W_tiles[ti]
                        else:
                            d, step = b["d"], b["step"]
                            ps = epsum.tile([128, HVE], f32, tag="ps")
                            for (tidx, qoff) in zip(b["tiles"], b["qoffs"]):
                                nc.tensor.matmul(
                                    out=ps[qoff:qoff + step, :],
                                    lhsT=cst(f"S{d}")[:, :step],
                                    rhs=W_tiles[tidx][:, :],
                                    start=True, stop=True)
                            nstrip = 2 if b["d"] == 2 else 3
                            for k2 in range(len(b["tiles"]), nstrip):
                                nc.tensor.matmul(
                                    out=ps[k2 * step:(k2 + 1) * step, :],
                                    lhsT=cst("Szero")[:, :step],
                                    rhs=W_tiles[b["tiles"][-1]][:, :],
                                    start=True, stop=True)
                            if b["d"] >= 4:
                                nc.vector.memset(ps[96:128, :], 0.0)
                            for tidx in b["tiles"]:
                                del W_tiles[tidx]
                            cpy(k, st["tile"][:, k, :], ps[:, :])
                        st["k"] += 1
                        bat_cursor[0] += 1
                        b0, b1 = sc_calls[sc_ci[0]]
                        if st["k"] == b1 - b0:
                            flush_scatter()

                idx_off = 0
                for ci, (t0, t1) in enumerate(g_calls):
                    ntl = t1 - t0
                    lo = tiles[t0][0] == 0
                    kvb = eg.tile([128, CN, 2 * IN], bf16, tag="kvb")
                    qab = eg.tile([128, CN, IN], bf16, tag="qab")
                    src_ap = (kv_tbl[0:LO_LIMIT, :] if lo
                              else kv_tbl[LO_LIMIT:NSLOT, :])
                    gi = nc.gpsimd.dma_gather(
                        kvb[:, :ntl, :], src_ap,
                        idx_kv[:, idx_off:idx_off + ntl * 8],
                        128 * ntl, 128 * ntl, 2 * IN, queue_num=0)
                    gate(gi, nf_nop)
                    add_dep_helper(gi.ins, lib_inst.ins, reason="lib")
                    gq = nc.gpsimd.dma_gather(
                        qab[:, :ntl, :], qa_tbl[:, :],
                        idx_qa[:, idx_off:idx_off + ntl * 8],
                        128 * ntl, 128 * ntl, IN, queue_num=0)
                    gate(gq, nf_nop)
                    add_dep_helper(gq.ins, lib_inst.ins, reason="lib")
                    idx_off += ntl * 8

                    ast = est.tile([128, CN, H], f32, tag="ast")
                    for j in range(ntl):
                        tmp = est.tile([128, 128], bf16, tag="tmp")
                        nc.vector.tensor_mul(out=tmp[:, :],
                                             in0=kvb[:, j, 0:IN],
                                             in1=qab[:, j, :])
                        nc.vector.reduce_sum(
                            out=ast[:, j, :],
                            in_=tmp[:, :].rearrange("p (h d) -> p h d", h=H),
                            axis=AX)
                    eat = est.tile([128, CN, H], bf16, tag="eat")
                    nc.scalar.activation(out=eat[:, :ntl, :],
                                         in_=ast[:, :ntl, :], func=AF.Exp,
                                         scale=1.0)
                    for j in range(ntl):
                        ti = t0 + j
                        Wt = ew.tile([128, HVE], bf16, tag="wt")
                        nc.vector.tensor_mul(
                            out=Wt[:, IN:HVE], in0=eat[:, j, :],
                            in1=msk[:, ti:ti + 1].broadcast_to([128, H]))
                        nc.vector.tensor_mul(
                            out=Wt[:, 0:IN].rearrange("p (h d) -> p h d", h=H),
                            in0=kvb[:, j, IN:2 * IN].rearrange(
                                "p (h d) -> p h d", h=H),
                            in1=expand_inner(Wt[:, IN:HVE], D))
                        W_tiles[ti] = Wt
                    process_ready_batches(t1)
                process_ready_batches(T + 1)
                flush_scatter()
                assert bat_cursor[0] == NB and not W_tiles
            # lo/hi serialization for hv scatter-adds
            if sc_lo and sc_hi:
                fn = fence(sc_lo)
                for si in sc_hi:
                    gate(si, fn)
            return sc_lo + sc_hi

        # ---------------- epilogue
        def epilogue(hv, xloc_tensor, out_tensor, out_dtype, sc_nop):
            outs = []
            with tc.tile_pool(name="epi", bufs=4) as epi, \
                 tc.tile_pool(name="epp", bufs=2, space="PSUM") as epp:
                for ch in range(NLOC // 128):
                    hvt = epi.tile([128, 4, HVROW], f32, tag="hvt")
                    ld = nc.sync.dma_start(
                        out=hvt[:, :, :],
                        in_=hv[ch * 512:(ch + 1) * 512, :].rearrange(
                            "(p g) f -> p g f", g=4))
                    gate(ld, sc_nop)
                    xl = epi.tile([128, 128], bf16, tag="xl")
                    ldx = nc.sync.dma_start(
                        out=xl[:, :],
                        in_=xloc_tensor[:, ch * 128:(ch + 1) * 128])
                    gate(ldx, sc_nop)
                    es = epi.tile([128, H], f32, tag="es")
                    e2 = epi.tile([128, H], f32, tag="e2")
                    nc.vector.tensor_add(out=es[:, :], in0=hvt[:, 0, IN:HVE],
                                         in1=hvt[:, 1, IN:HVE])
                    nc.vector.tensor_add(out=e2[:, :], in0=hvt[:, 2, IN:HVE],
                                         in1=hvt[:, 3, IN:HVE])
                    nc.vector.tensor_add(out=es[:, :], in0=es[:, :],
                                         in1=e2[:, :])
                    nc.vector.tensor_scalar_add(out=es[:, :], in0=es[:, :],
                                                scalar1=EPS_ESUM)
                    nc.vector.reciprocal(out=es[:, :], in_=es[:, :])
                    t = int(slot_t[ch * 128])
                    ops = epp.tile([128, 128], f32, tag="ops")
                    for g in range(ET):
                        hvn = epi.tile([128, 128], bf16, tag="hvn")
                        nc.vector.tensor_mul(
                            out=hvn[:, :].rearrange("p (h d) -> p h d", h=H),
                            in0=hvt[:, g, 0:IN].rearrange(
                                "p (h d) -> p h d", h=H),
                            in1=expand_inner(es[:, :], D))
                        tp = epp.tile([128, 128], bf16, tag="tp")
                        nc.tensor.transpose(out=tp[:, :], in_=hvn[:, :],
                                            identity=cst("ident"))
                        hvnT = epi.tile([128, 128], bf16, tag="hvnT")
                        cpy(g, hvnT[:, :], tp[:, :])
                        nc.tensor.matmul(out=ops[:, :], lhsT=cst(f"M2{g}{t}"),
                                         rhs=hvnT[:, :], start=(g == 0),
                                         stop=(g == 3))
                    dd = epi.tile([128, 128], f32, tag="dd")
                    nc.vector.tensor_sub(out=dd[:, :], in0=ops[:, :],
                                         in1=xl[:, :])
                    nc.vector.tensor_scalar_mul(out=dd[:, :], in0=dd[:, :],
                                                scalar1=float(alpha[t]))
                    ot = epi.tile([128, 128], out_dtype, tag="ot")
                    nc.vector.tensor_add(out=ot[:, :], in0=dd[:, :],
                                         in1=xl[:, :])
                    outs.append(nc.sync.dma_start(
                        out=out_tensor[:, ch * 128:(ch + 1) * 128],
                        in_=ot[:, :]))
            return outs

        # ================= single layer (optionally repeated for timing)
        prev = None
        for rep in range(repeats):
            if rep > 0:
                zh1 = zero_hv(hv1)
                for z in zh1:
                    gate(z, prev)
            writes1 = []
            node_front(xT_in[:, :], xTloc_in, prev, writes1)
            f1 = fence(writes1)
            sc1 = edge_phase(hv1, f1, zh1)
            f3 = fence(sc1)
            eps = epilogue(hv1, xTloc_in, out_t, out_dt, f3)
            prev = fence(eps)

    nc.compile()
    return nc


# ---------------------------------------------------------------- runner
def _in_maps(meta, percore, shared, xT, xTloc_percore):
    maps = []
    for c in range(NCORES):
        maps.append({
            "xT": xT, "xTloc": xTloc_percore[c],
            "wconst": shared["wconst"], "kvidx": percore["kv_idx"][c],
            "qaidx": percore["qa_idx"][c], "scidx": percore["sc_idx"][c],
            "mask": percore["mask"][c]})
    return maps


def kernel(**inputs) -> np.ndarray:
    from concourse import bass2jax

    meta, percore, shared, asm = _host_prep(inputs)
    nc1 = _build(meta, last_layer=False)
    nc2 = _build(meta, last_layer=True)

    maps1 = _in_maps(meta, percore, shared, shared["xT"], percore["xTloc"])
    res1 = bass2jax.run_bass_via_pjrt(nc1, maps1, n_cores=NCORES)
    xnew = [np.asarray(res1[c]["out"]) for c in range(NCORES)]   # (128,6400) bf16
    xag = np.concatenate(xnew, 0)                                # (1024,6400)

    maps2 = _in_maps(meta, percore, shared, xag, xnew)
    res2 = bass2jax.run_bass_via_pjrt(nc2, maps2, n_cores=NCORES)

    out = np.zeros((N, IN), np.float32)
    for c in range(NCORES):
        oc = np.asarray(res2[c]["out"]).T
        sl = slice(c * NLOC, (c + 1) * NLOC)
        rl = asm["real"][sl]
        out[asm["old_of_slot"][sl][rl]] = oc[rl]
    return out

